# revision 28
# baseline (speedup 1.0000x reference)
"""Multi-head causal self-attention (B=4, T=2048, D=1024, H=16) on 8 TRN2 cores.

Sharding (hardcoded): data-parallel over the 4 batches x tensor-parallel over
head halves. Core c handles batch c//2 and local heads (c%2)*8 .. (c%2)*8+7
for all 2048 positions. Every core runs the same SPMD program on its slice:

  x[b] [2048,1024] -> x^T in SBUF via PE-transpose (bf16 cast on psum copy)
  Q^T = (Wq_slice)^T x^T / 8,  K^T = (Wk_slice)^T x^T       [dh-pairs packed
  V   = x Wv_slice (+ ones column for the softmax denominator)   on 128 parts]
  S^T = K Q^T per 128x512 block (four query blocks share one matmul; causal
        blocks only; head pairs run concurrently via PE row groups; future
        positions get -1e9 via triangular/full-mask matmul accumulands),
  P^T = exp(S^T) (ScalarE, straight from PSUM, bf16 out),
  ctx^T = V^T P (V is the stationary operand, so ctx comes out transposed;
        row 64 of the product is the softmax denominator l),
  ctx^T *= 1/l (reciprocal + partition-broadcast DMA + multiply),
  partial_out = ctx^T.T @ Wo_slice.

The host sums the two partial outputs per batch and adds the bias bo.
"""
import numpy as np

import concourse.bass as bass
import concourse.mybir as mybir
import concourse.tile as tile
from concourse import bacc
from concourse.bass_utils import run_bass_kernel_spmd
from concourse.masks import make_identity, make_lower_triangular

F32 = mybir.dt.float32
BF16 = mybir.dt.bfloat16
AF = mybir.ActivationFunctionType

B, T, D = 4, 2048, 1024
HL = 8              # local heads per core
HP = HL // 2        # local head pairs (two heads share 128 partitions)
DH = 64
PO = D // 128       # contraction chunks over D
CD = HL * DH        # 512: local context feature dim
FC = CD // 128      # 4
NB = T // 128       # 16 query/key blocks of 128
QUAD = 4            # query blocks handled together (512 S^T columns)
SCALE = 1.0 / 8.0   # 1/sqrt(DH)
NEG = -1e9
CHUNK = 2           # key blocks per S^T psum tile ([128, 2*512] = 2 banks)


def _emit_attention_quad(nc, qb0, kt_sb, qt_sb, v_sb, ident, utri, negf,
                         ptp, stp, cxp, lvp, ctp, dramp, ctxt16s):
    """Attention for query blocks qb0..qb0+3, all 4 local head pairs.

    For each key block kb one N=512 matmul covers all four query blocks.
    Future (k > q) positions are pushed to -1e9 by accumulating mask
    matmuls (identity^T @ utri / negf), so exp() zeroes them and they
    drop out of both the context and the denominator.
    """
    nkb = qb0 + QUAD   # key blocks needed (for the last query block)
    qsl = slice(qb0 * 128, (qb0 + QUAD) * 128)
    for hp in range(HP):
        ctxT = {}
        for par in (0, 1):
            ctxT[par] = cxp.tile([128, 512], F32, tag="cx", name=f"ctxT{par}")
        nchunks = (nkb + CHUNK - 1) // CHUNK
        for ch in range(nchunks):
            k0 = ch * CHUNK
            k1 = min(nkb, k0 + CHUNK)
            st_e = stp.tile([128, 512 * CHUNK], F32, tag="st")
            st_o = stp.tile([128, 512 * CHUNK], F32, tag="st")
            pt_e = ptp.tile([128, CHUNK, 512], BF16, tag="pt")
            pt_o = ptp.tile([128, CHUNK, 512], BF16, tag="pt")
            for kb in range(k0, k1):
                w = (kb - k0) * 512
                j = kb - qb0   # diagonal column index within the quad
                for st, lo in ((st_e, 0), (st_o, 64)):
                    last = kb < qb0  # no mask matmuls needed
                    nc.tensor.matmul(
                        st[:, w:w + 512],
                        lhsT=kt_sb[lo:lo + 64, hp, kb * 128:(kb + 1) * 128],
                        rhs=qt_sb[lo:lo + 64, hp, qsl],
                        start=True, stop=last,
                    )
                    if kb >= qb0:
                        if j > 0:   # fully mask query blocks left of the diag
                            nc.tensor.matmul(
                                st[:, w:w + j * 128],
                                lhsT=ident, rhs=negf[:, :j * 128],
                                start=False, stop=False,
                            )
                        # strict upper triangle of the diagonal block
                        nc.tensor.matmul(
                            st[:, w + j * 128:w + (j + 1) * 128],
                            lhsT=ident, rhs=utri, start=False, stop=True,
                        )
            ncol = (k1 - k0) * 512
            nc.scalar.activation(pt_e[:, :k1 - k0, :], st_e[:, :ncol], AF.Exp)
            nc.scalar.activation(pt_o[:, :k1 - k0, :], st_o[:, :ncol], AF.Exp)
            for kb in range(k0, k1):
                for par, pt in ((0, pt_e), (1, pt_o)):
                    nc.tensor.matmul(
                        ctxT[par][0:65, :],
                        lhsT=v_sb[:, kb, 2 * hp + par, :],
                        rhs=pt[:, kb - k0, :],
                        start=(kb == 0), stop=(kb == nkb - 1),
                    )
        # normalize: ctx^T[dh, q] /= l[q] (l sits in row 64 of the product).
        # l is replicated across 64 partitions via a DRAM-bounce broadcast
        # DMA, then inverted and multiplied in.
        ctxt16 = ctxt16s[hp]
        for par in (0, 1):
            lrow = lvp.tile([65, 512], F32, tag="lrow")
            nc.vector.tensor_copy(lrow[64:65, :], ctxT[par][64:65, :])
            ldram = dramp.tile([1, 512], F32, tag="ldram")
            nc.sync.dma_start(ldram, lrow[64:65, :])
            lb = lvp.tile([64, 512], F32, tag="lb")
            bcast = bass.AP(
                tensor=ldram.tensor, offset=ldram.offset,
                ap=[[0, 64]] + [list(a) for a in ldram.ap[1:]])
            nc.sync.dma_start(lb, bcast)
            linvb = lvp.tile([64, 512], F32, tag="linvb")
            nc.vector.reciprocal(linvb, lb)
            if par == 0:
                nc.vector.tensor_mul(ctxt16[0:64, :], ctxT[par][0:64, :], linvb)
            else:
                tmp = ctp.tile([64, 512], BF16, tag="ctmp")
                nc.vector.tensor_mul(tmp, ctxT[par][0:64, :], linvb)
                nc.sync.dma_start(ctxt16[64:128, :], tmp)


def _emit_out_proj_qb(nc, qb0, qloc, ctxt16s, wo_sb, mmp, osbp, out_d):
    """Output projection for query block qb0 + qloc."""
    qb = qb0 + qloc
    for dw in range(2):
        ps = mmp.tile([128, 512], F32, tag="mm")
        for hp in range(HP):
            nc.tensor.matmul(
                ps, lhsT=ctxt16s[hp][:, qloc * 128:(qloc + 1) * 128],
                rhs=wo_sb[:, hp, dw * 512:(dw + 1) * 512],
                start=(hp == 0), stop=(hp == HP - 1),
            )
        osb = osbp.tile([128, 512], F32, tag="osb")
        nc.vector.tensor_copy(osb, ps)
        nc.sync.dma_start(out_d[qb * 128:(qb + 1) * 128, dw * 512:(dw + 1) * 512], osb)


def _emit_xt_tb(nc, tb, x_d, identf, xstage, xt_sb, mmp):
    """One 128-row block of x -> x^T: f32 load, 8 PE transposes, bf16 copies."""
    xf = xstage.tile([128, D], F32, tag="xf")
    nc.sync.dma_start(xf, x_d[tb * 128:(tb + 1) * 128, :])
    for po in range(PO):
        tp = mmp.tile([128, 128], F32, tag="mm", name="xtp")
        nc.tensor.transpose(tp, xf[:, po * 128:(po + 1) * 128], identf)
        if po % 2 == 0:
            nc.vector.tensor_copy(xt_sb[:, po, tb * 128:(tb + 1) * 128], tp)
        else:
            nc.scalar.activation(xt_sb[:, po, tb * 128:(tb + 1) * 128], tp,
                                 AF.Copy)


def _emit_proj_tw(nc, tw, xt_sb, mmp, wq_sb, wk_sb, wv_sb, kt_sb, qt_sb, v_sb):
    """Project K^T, Q^T, V for one 512-column group of x^T."""
    tsl = slice(tw * 512, (tw + 1) * 512)
    for hp in range(HP):
        ps = mmp.tile([128, 512], F32, tag="mm")
        for po in range(PO):
            nc.tensor.matmul(
                ps, lhsT=wk_sb[:, po, hp * 128:(hp + 1) * 128], rhs=xt_sb[:, po, tsl],
                start=(po == 0), stop=(po == PO - 1),
            )
        nc.vector.tensor_copy(kt_sb[:, hp, tsl], ps)
    for hp in range(HP):
        ps = mmp.tile([128, 512], F32, tag="mm")
        for po in range(PO):
            nc.tensor.matmul(
                ps, lhsT=wq_sb[:, po, hp * 128:(hp + 1) * 128], rhs=xt_sb[:, po, tsl],
                start=(po == 0), stop=(po == PO - 1),
            )
        nc.scalar.activation(qt_sb[:, hp, tsl], ps, AF.Copy, scale=SCALE)
    for tb in range(4):
        kb = tw * 4 + tb
        ps = mmp.tile([128, 512], F32, tag="mm")
        for po in range(PO):
            nc.tensor.matmul(
                ps, lhsT=xt_sb[:, po, kb * 128:(kb + 1) * 128], rhs=wv_sb[:, po, :],
                start=(po == 0), stop=(po == PO - 1),
            )
        nc.vector.tensor_copy(
            v_sb[:, kb, :, 0:64], ps.rearrange("p (h d) -> p h d", h=HL)
        )


def build_nc():
    nc = bacc.Bacc("TRN2", target_bir_lowering=False)
    x_d = nc.dram_tensor("x", [T, D], F32, kind="ExternalInput")
    wq_d = nc.dram_tensor("wq", [D, CD], F32, kind="ExternalInput")
    wk_d = nc.dram_tensor("wk", [D, CD], F32, kind="ExternalInput")
    wv_d = nc.dram_tensor("wv", [D, CD], F32, kind="ExternalInput")
    wo_d = nc.dram_tensor("wo", [CD, D], F32, kind="ExternalInput")
    out_d = nc.dram_tensor("out", [T, D], F32, kind="ExternalOutput")

    with tile.TileContext(nc) as tc:
        with (
            tc.tile_pool(name="consts", bufs=1) as consts,
            tc.tile_pool(name="wsb", bufs=1) as wsb,
            tc.tile_pool(name="wstage", bufs=1) as wstage,
            tc.tile_pool(name="xstage", bufs=3) as xstage,
            tc.tile_pool(name="big", bufs=1) as big,
            tc.tile_pool(name="pt", bufs=6) as ptp,
            tc.tile_pool(name="lv", bufs=4) as lvp,
            tc.tile_pool(name="ct", bufs=2) as ctp,
            tc.tile_pool(name="ctxt16", bufs=8) as ctxt16p,
            tc.tile_pool(name="osb", bufs=2) as osbp,
            tc.tile_pool(name="dram", bufs=4, space="DRAM") as dramp,
            tc.tile_pool(name="mm", bufs=2, space="PSUM") as mmp,
            tc.tile_pool(name="st", bufs=2, space="PSUM") as stp,
            tc.tile_pool(name="cx", bufs=2, space="PSUM") as cxp,
        ):
            ident = consts.tile([128, 128], BF16, tag="ident")
            make_identity(nc, ident)
            identf = consts.tile([128, 128], F32, tag="identf")
            make_identity(nc, identf)
            utri = consts.tile([128, 128], BF16, tag="utri")
            make_lower_triangular(nc, utri, val=NEG, diag=False)
            negf = consts.tile([128, 384], BF16, tag="negf")
            nc.gpsimd.memset(negf, NEG)

            wq_sb = wsb.tile([128, PO, CD], BF16, tag="wq")
            wk_sb = wsb.tile([128, PO, CD], BF16, tag="wk")
            wv_sb = wsb.tile([128, PO, CD], BF16, tag="wv")
            wo_sb = wsb.tile([128, FC, D], BF16, tag="wo")
            for i, (dram, sb, shp) in enumerate((
                (wq_d, wq_sb, (PO, CD)),
                (wk_d, wk_sb, (PO, CD)),
                (wv_d, wv_sb, (PO, CD)),
                (wo_d, wo_sb, (FC, D)),
            )):
                stg = wstage.tile([128, shp[0], shp[1]], F32, tag="ws")
                nc.sync.dma_start(stg, dram.rearrange("(po p) n -> p po n", p=128))
                if i % 2 == 0:
                    nc.vector.tensor_copy(sb, stg)
                else:
                    nc.scalar.activation(sb, stg, AF.Copy)

            xt_sb = big.tile([128, PO, T], BF16, tag="xt")
            kt_sb = big.tile([128, HP, T], BF16, tag="kt")
            qt_sb = big.tile([128, HP, T], BF16, tag="qt")
            v_sb = big.tile([128, NB, HL, 65], BF16, tag="v")
            nc.gpsimd.memset(v_sb[:, :, :, 64:65], 1.0)

            # all of x^T upfront: warms the PE and removes every DMA
            # dependency from the proj/attention phases
            for tb in range(NB):
                _emit_xt_tb(nc, tb, x_d, identf, xstage, xt_sb, mmp)

            for half in range(2):
                for tw in range(2 * half, 2 * half + 2):
                    _emit_proj_tw(nc, tw, xt_sb, mmp,
                                  wq_sb, wk_sb, wv_sb, kt_sb, qt_sb, v_sb)
                for qb0 in range(8 * half, 8 * half + 8, QUAD):
                    ctxt16s = [ctxt16p.tile([128, 512], BF16, tag="c16",
                                            name=f"c16_{hp}")
                               for hp in range(HP)]
                    _emit_attention_quad(nc, qb0, kt_sb, qt_sb, v_sb, ident,
                                         utri, negf, ptp, stp, cxp, lvp, ctp,
                                         dramp, ctxt16s)
                    for qloc in range(QUAD):
                        _emit_out_proj_qb(nc, qb0, qloc, ctxt16s, wo_sb,
                                          mmp, osbp, out_d)

    nc.compile()
    return nc


_CACHE = {}


def _get_nc():
    if "nc" not in _CACHE:
        _CACHE["nc"] = build_nc()
    return _CACHE["nc"]


def make_in_maps(x, Wq, Wk, Wv, Wo):
    x = np.asarray(x, np.float32)
    Wq = np.asarray(Wq, np.float32)
    Wk = np.asarray(Wk, np.float32)
    Wv = np.asarray(Wv, np.float32)
    Wo = np.asarray(Wo, np.float32)
    in_maps = []
    for c in range(8):
        b, hh = c // 2, c % 2
        cols = slice(hh * CD, (hh + 1) * CD)
        in_maps.append({
            "x": np.ascontiguousarray(x[b]),
            "wq": np.ascontiguousarray(Wq[:, cols]),
            "wk": np.ascontiguousarray(Wk[:, cols]),
            "wv": np.ascontiguousarray(Wv[:, cols]),
            "wo": np.ascontiguousarray(Wo[cols, :]),
        })
    return in_maps


def gather_output(results, bo):
    bo = np.asarray(bo, np.float32)
    out = np.empty((B, T, D), np.float32)
    for b in range(B):
        out[b] = results[2 * b]["out"] + results[2 * b + 1]["out"] + bo[None, :]
    return out


def kernel(x, Wq, Wk, Wv, Wo, bo):
    nc = _get_nc()
    in_maps = make_in_maps(x, Wq, Wk, Wv, Wo)
    res = run_bass_kernel_spmd(nc, in_maps, core_ids=list(range(8)))
    return gather_output(res.results, bo)
